# revision 6
# baseline (speedup 1.0000x reference)
"""Trainium2 Bass kernel for batched per-item GRU cell (bf16 PE pipeline).

Problem: nn_GRU_Cell — B=16, N=207 independent items, each with its own
C=64 -> 3H=192 weight matrices (Wx, Wh).  All ops are per-(b,n):

    xW          = x @ Wx                      [1, 192]
    r           = sigmoid(xW_r + h @ Wh_r + b_r)
    z           = sigmoid(xW_z + h @ Wh_z + b_z)
    hc          = tanh  (xW_c + (r*h) @ Wh_c + b_c)
    h_new       = (1 - z) * h + z * hc

Strategy (per core, items sharded 3312 -> 8 x 414):
  * Weights are both the DMA and the PE bottleneck.  They stream ONCE as
    bf16 (halves HBM traffic vs f32, and bf16 stationaries load with the
    PE's Fast-Weight-Load at ~2x; f32 matmuls would also double-pass).
    Accuracy: bf16 weights/activations with f32 PSUM accumulate gives
    rel-err ~1.8e-3 on this problem (gate is 2e-2).
  * Per item the weights are the PE *stationary* operand, K-stacked:
      S_rz       = [Wx[:, 0:128] ; Wh[:, 0:128]]    (K=128, M=128)
      S_c (pair) = [Wc_even | Wc_odd], Wc = [Wx[:,128:192]; Wh[:,128:192]]
    The c-weights of two adjacent items are packed into ONE 128-column
    stationary so every LDWEIGHTS is a full 128-column load (FWL-eligible)
    and two c-matmuls share one weight load.
  * Moving operands are single bf16 columns:
      rz: [x ; h]     -> psum_rz[:, g]  (r rows 0:64, z rows 64:128)
      c : [x ; r*h]   -> psum_c [:, g]  (even items' c in rows 0:64,
                                         odd items' c in rows 64:128)
  * All input transposition is done HOST-side: x/h/biases arrive as
    [feature, item] panels, so the kernel issues ZERO PE transposes and
    no ACT staging copies.  The output stays [H, items]; the host
    transposes it back.
  * The c-pass of chunk k is issued after the rz-pass of chunk k+1, so
    the PE never waits on the sigmoid/DVE producing the c moving operand.
"""

import numpy as np

import concourse.bass as bass
import concourse.mybir as mybir
import concourse.tile as tile
from concourse import bacc
from concourse.bass_utils import run_bass_kernel_spmd

F32 = mybir.dt.float32
BF16 = mybir.dt.bfloat16
AF = mybir.ActivationFunctionType

B, N, C, H = 16, 207, 64, 64
J = 3 * H                  # 192
ITEMS = B * N              # 3312
NCORES = 8
PER = ITEMS // NCORES      # 414
# Small first chunk so the PE starts (and buffer recycling begins) early;
# small last chunks so the post-DMA tail is short.  All even (c-pass pairs).
CHUNKS = [16] + [52] * 7 + [18] + [10] + [6]   # sum = 414
NCHUNK = len(CHUNKS)
GMAX = max(CHUNKS)


def build_nc():
    nc = bacc.Bacc(None)
    # host-pre-transposed activation panels
    #   xhx  bf16 [128, 2*PER]: cols 0:PER = [x;h], cols PER:2PER = [x;0]
    #        (rows 64:128 of the second block get r*h written on-chip)
    #   auxf f32  [128, 3*PER]: block0 rows 64:128 = h, block1 = b_rz^T,
    #        block2 = b_c duplicated (even cols rows 0:64, odd rows 64:128)
    xhx_d = nc.declare_dram_parameter("xhx", [128, 2 * PER], BF16,
                                      isOutput=False)
    auxf_d = nc.declare_dram_parameter("auxf", [128, 3 * PER], F32,
                                       isOutput=False)
    # per-chunk blocks, each [c=128, G*192] flattened: G rz-stationaries
    # (128 cols each) then G/2 paired c-stationaries (128 cols each)
    w_d = nc.declare_dram_parameter("wxh", [PER * 2 * C * J], BF16,
                                    isOutput=False)
    out_d = nc.declare_dram_parameter("out", [64, PER], F32, isOutput=True)

    with tile.TileContext(nc) as tc:
        with (
            tc.tile_pool(name="const", bufs=1) as cpool,
            tc.tile_pool(name="w", bufs=6) as wpool,
            tc.tile_pool(name="ep", bufs=2) as epool,
            tc.tile_pool(name="prz", bufs=3, space="PSUM") as prz_pool,
            tc.tile_pool(name="pc", bufs=3, space="PSUM") as pc_pool,
        ):
            # preloads go on the SWDGE queue so both HWDGE queues carry
            # nothing but the back-to-back weight stream
            xhx = cpool.tile([128, 2 * PER], BF16)
            nc.gpsimd.dma_start(out=xhx[:], in_=xhx_d[:])
            auxf = cpool.tile([128, 3 * PER], F32)
            nc.gpsimd.dma_start(out=auxf[:], in_=auxf_d[:])
            hn = cpool.tile([128, PER], F32)

            h_f = auxf[:, 0:PER]               # rows 64:128 = h (f32)
            b_rz = auxf[:, PER:2 * PER]
            bc2 = auxf[:, 2 * PER:3 * PER]
            rhs2 = xhx[:, PER:2 * PER]         # rows 0:64 = x (bf16)

            # software pipeline state: chunk k's c-pass+epilogue runs after
            # chunk k+1's rz-pass so PE never stalls on the sigmoid chain
            pending = None

            def rz_pass(k, s, G):
                w = wpool.tile([128, GMAX * J], BF16, tag="w")
                wq = nc.sync if k % 2 == 0 else nc.scalar
                wq.dma_start(
                    out=w[:, 0:G * J],
                    in_=w_d[s * 128 * J:(s + G) * 128 * J].rearrange(
                        "(c v) -> c v", c=128),
                )
                psum_rz = prz_pool.tile([128, GMAX], F32, tag="rz")
                for g in range(G):
                    nc.tensor.matmul(
                        psum_rz[:, g:g + 1],
                        w[:, g * 128:(g + 1) * 128],
                        xhx[:, s + g:s + g + 1],
                        start=True, stop=True,
                    )
                # r/z and the c-pass moving columns
                t_rz = epool.tile([128, GMAX], F32, tag="t_rz")
                nc.vector.tensor_add(t_rz[:, 0:G], psum_rz[:, 0:G],
                                     b_rz[:, s:s + G])
                rs = epool.tile([128, GMAX], F32, tag="rs")
                nc.scalar.activation(rs[64:128, 0:G], t_rz[0:64, 0:G],
                                     AF.Sigmoid)
                zs = epool.tile([128, GMAX], F32, tag="zs")
                nc.scalar.activation(zs[64:128, 0:G], t_rz[64:128, 0:G],
                                     AF.Sigmoid)
                nc.vector.tensor_mul(rhs2[64:128, s:s + G], rs[64:128, 0:G],
                                     h_f[64:128, s:s + G])
                return w, zs

            def c_pass(k, s, G, w, zs):
                psum_c = pc_pool.tile([128, GMAX], F32, tag="c")
                cbase = G * 128
                for t in range(G // 2):
                    lw = w[:, cbase + t * 128:cbase + (t + 1) * 128]
                    nc.tensor.matmul(
                        psum_c[:, 2 * t:2 * t + 1], lw,
                        rhs2[:, s + 2 * t:s + 2 * t + 1],
                        start=True, stop=True,
                    )
                    nc.tensor.matmul(
                        psum_c[:, 2 * t + 1:2 * t + 2], lw,
                        rhs2[:, s + 2 * t + 1:s + 2 * t + 2],
                        start=True, stop=True,
                    )
                t_c = epool.tile([128, GMAX], F32, tag="t_c")
                nc.vector.tensor_add(t_c[:, 0:G], psum_c[:, 0:G],
                                     bc2[:, s:s + G])
                # even items' c sits in rows 0:64, odd items' in 64:128
                hc = epool.tile([128, GMAX], F32, tag="hc")
                nc.scalar.activation(hc[64:128, 0:G:2], t_c[0:64, 0:G:2],
                                     AF.Tanh)
                nc.scalar.activation(hc[64:128, 1:G:2], t_c[64:128, 1:G:2],
                                     AF.Tanh)
                # h_new = h + z*(hc - h)
                diff = epool.tile([128, GMAX], F32, tag="diff")
                nc.vector.tensor_sub(diff[64:128, 0:G], hc[64:128, 0:G],
                                     h_f[64:128, s:s + G])
                prod = epool.tile([128, GMAX], F32, tag="prod")
                nc.vector.tensor_mul(prod[64:128, 0:G], zs[64:128, 0:G],
                                     diff[64:128, 0:G])
                nc.vector.tensor_add(hn[64:128, s:s + G],
                                     h_f[64:128, s:s + G],
                                     prod[64:128, 0:G])
                # HWDGE store: the sync queue is idle between weight chunks
                # and at kernel end (SWDGE would add ~1us descriptor-gen +
                # ~2us completion to the final store's critical path)
                nc.sync.dma_start(out=out_d[:, s:s + G],
                                  in_=hn[64:128, s:s + G])

            s = 0
            for k in range(NCHUNK):
                G = CHUNKS[k]
                state = rz_pass(k, s, G)
                if pending is not None:
                    c_pass(*pending)
                pending = (k, s, G) + state
                s += G
            c_pass(*pending)

    nc.compile()
    return nc


_CACHE = {}


def _get_nc():
    if "nc" not in _CACHE:
        _CACHE["nc"] = build_nc()
    return _CACHE["nc"]


def _pack(x, state, Wx, Wh, b):
    import ml_dtypes
    BF = ml_dtypes.bfloat16
    x2 = np.asarray(x, np.float32).reshape(ITEMS, C)
    h2 = np.asarray(state, np.float32).reshape(ITEMS, H)
    b2 = np.asarray(b, np.float32).reshape(ITEMS, J)
    wx = np.asarray(Wx, np.float32).reshape(ITEMS, C, J)
    wh = np.asarray(Wh, np.float32).reshape(ITEMS, H, J)
    w2 = np.concatenate([wx, wh], axis=1).astype(BF)   # [ITEMS, 128, 192]
    maps = []
    for i in range(NCORES):
        sl = slice(i * PER, (i + 1) * PER)
        xi, hi, bi, wi = x2[sl], h2[sl], b2[sl], w2[sl]

        xhx = np.zeros((128, 2 * PER), BF)
        xhx[0:64, 0:PER] = xi.T
        xhx[64:128, 0:PER] = hi.T
        xhx[0:64, PER:2 * PER] = xi.T

        auxf = np.zeros((128, 3 * PER), np.float32)
        auxf[64:128, 0:PER] = hi.T
        auxf[:, PER:2 * PER] = bi[:, 0:128].T
        bc = bi[:, 128:192].T                    # [64, PER]
        auxf[0:64, 2 * PER + 0:3 * PER:2] = bc[:, 0::2]
        auxf[64:128, 2 * PER + 1:3 * PER:2] = bc[:, 1::2]

        blocks = []
        s = 0
        for G in CHUNKS:
            wc = wi[s:s + G]                               # [G, 128, 192]
            rz = wc[:, :, 0:128].transpose(1, 0, 2).reshape(128, G * 128)
            cc = wc[:, :, 128:192].transpose(1, 0, 2).reshape(128, G * 64)
            blocks.append(
                np.ascontiguousarray(
                    np.concatenate([rz, cc], axis=1)).reshape(-1))
            s += G
        maps.append({"xhx": xhx, "auxf": auxf,
                     "wxh": np.concatenate(blocks)})
    return maps


def kernel(x, state, Wx, Wh, b, _trace=False):
    nc = _get_nc()
    in_maps = _pack(x, state, Wx, Wh, b)
    res = run_bass_kernel_spmd(nc, in_maps, list(range(NCORES)), trace=_trace)
    out = np.concatenate(
        [res.results[i]["out"].T for i in range(NCORES)], axis=0)
    ret = np.ascontiguousarray(out.reshape(B, N, 1, H), dtype=np.float32)
    if _trace:
        return ret, res
    return ret


# revision 9
# speedup vs baseline: 1.0083x; 1.0083x over previous
"""Trainium2 Bass kernel for batched per-item GRU cell (bf16 PE pipeline).

Problem: nn_GRU_Cell — B=16, N=207 independent items, each with its own
C=64 -> 3H=192 weight matrices (Wx, Wh).  All ops are per-(b,n):

    xW          = x @ Wx                      [1, 192]
    r           = sigmoid(xW_r + h @ Wh_r + b_r)
    z           = sigmoid(xW_z + h @ Wh_z + b_z)
    hc          = tanh  (xW_c + (r*h) @ Wh_c + b_c)
    h_new       = (1 - z) * h + z * hc

Strategy (per core, items sharded 3312 -> 8 x 414):
  * Weights are both the DMA and the PE bottleneck.  They stream ONCE as
    bf16 (halves HBM traffic vs f32, and bf16 stationaries load with the
    PE's Fast-Weight-Load at ~2x; f32 matmuls would also double-pass).
    Accuracy: bf16 weights/activations with f32 PSUM accumulate gives
    rel-err ~1.8e-3 on this problem (gate is 2e-2).
  * Per item the weights are the PE *stationary* operand, K-stacked:
      S_rz       = [Wx[:, 0:128] ; Wh[:, 0:128]]    (K=128, M=128)
      S_c (pair) = [Wc_even | Wc_odd], Wc = [Wx[:,128:192]; Wh[:,128:192]]
    The c-weights of two adjacent items are packed into ONE 128-column
    stationary so every LDWEIGHTS is a full 128-column load (FWL-eligible)
    and two c-matmuls share one weight load.
  * Moving operands are single bf16 columns:
      rz: [x ; h]     -> psum_rz[:, g]  (r rows 0:64, z rows 64:128)
      c : [x ; r*h]   -> psum_c [:, g]  (even items' c in rows 0:64,
                                         odd items' c in rows 64:128)
  * All input transposition is done HOST-side: x/h/biases arrive as
    [feature, item] panels, so the kernel issues ZERO PE transposes and
    no ACT staging copies.  The output stays [H, items]; the host
    transposes it back.
  * The c-pass of chunk k is issued after the rz-pass of chunk k+1, so
    the PE never waits on the sigmoid/DVE producing the c moving operand.
"""

import numpy as np

import concourse.bass as bass
import concourse.mybir as mybir
import concourse.tile as tile
from concourse import bacc
from concourse.bass_utils import run_bass_kernel_spmd

F32 = mybir.dt.float32
BF16 = mybir.dt.bfloat16
AF = mybir.ActivationFunctionType

B, N, C, H = 16, 207, 64, 64
J = 3 * H                  # 192
ITEMS = B * N              # 3312
NCORES = 8
PER = ITEMS // NCORES      # 414
# Small first chunk so the PE starts (and buffer recycling begins) early;
# small last chunks so the post-DMA tail is short.  All even (c-pass pairs).
CHUNKS = [16] + [52] * 7 + [22] + [12]   # sum = 414
NCHUNK = len(CHUNKS)
GMAX = max(CHUNKS)


def build_nc():
    nc = bacc.Bacc(None)
    # host-pre-transposed activation panels
    #   xhx  bf16 [128, 2*PER]: cols 0:PER = [x;h], cols PER:2PER = [x;0]
    #        (rows 64:128 of the second block get r*h written on-chip)
    #   auxf f32  [128, 3*PER]: block0 rows 64:128 = h, block1 = b_rz^T,
    #        block2 = b_c duplicated (even cols rows 0:64, odd rows 64:128)
    xhx_d = nc.declare_dram_parameter("xhx", [128, 2 * PER], BF16,
                                      isOutput=False)
    auxf_d = nc.declare_dram_parameter("auxf", [128, 3 * PER], F32,
                                       isOutput=False)
    # per-chunk blocks, each [c=128, G*192] flattened: G rz-stationaries
    # (128 cols each) then G/2 paired c-stationaries (128 cols each)
    w_d = nc.declare_dram_parameter("wxh", [PER * 2 * C * J], BF16,
                                    isOutput=False)
    out_d = nc.declare_dram_parameter("out", [64, PER], F32, isOutput=True)

    with tile.TileContext(nc) as tc:
        with (
            tc.tile_pool(name="const", bufs=1) as cpool,
            tc.tile_pool(name="w", bufs=6) as wpool,
            tc.tile_pool(name="ep", bufs=2) as epool,
            tc.tile_pool(name="prz", bufs=3, space="PSUM") as prz_pool,
            tc.tile_pool(name="pc", bufs=3, space="PSUM") as pc_pool,
        ):
            # one small preload leads each HWDGE queue (SWDGE would crawl at
            # ~65GB/s and burn ~25us of GpSimd Q7 descriptor generation)
            xhx = cpool.tile([128, 2 * PER], BF16)
            nc.sync.dma_start(out=xhx[:], in_=xhx_d[:])
            auxf = cpool.tile([128, 3 * PER], F32)
            nc.scalar.dma_start(out=auxf[:], in_=auxf_d[:])
            hn = cpool.tile([128, PER], F32)

            h_f = auxf[:, 0:PER]               # rows 64:128 = h (f32)
            b_rz = auxf[:, PER:2 * PER]
            bc2 = auxf[:, 2 * PER:3 * PER]
            rhs2 = xhx[:, PER:2 * PER]         # rows 0:64 = x (bf16)

            # software pipeline state: chunk k's c-pass+epilogue runs after
            # chunk k+1's rz-pass so PE never stalls on the sigmoid chain
            pending = None

            def rz_pass(k, s, G):
                w = wpool.tile([128, GMAX * J], BF16, tag="w")
                wq = nc.sync if k % 2 == 0 else nc.scalar
                wq.dma_start(
                    out=w[:, 0:G * J],
                    in_=w_d[s * 128 * J:(s + G) * 128 * J].rearrange(
                        "(c v) -> c v", c=128),
                )
                psum_rz = prz_pool.tile([128, GMAX], F32, tag="rz")
                for g in range(G):
                    nc.tensor.matmul(
                        psum_rz[:, g:g + 1],
                        w[:, g * 128:(g + 1) * 128],
                        xhx[:, s + g:s + g + 1],
                        start=True, stop=True,
                    )
                # r/z and the c-pass moving columns
                t_rz = epool.tile([128, GMAX], F32, tag="t_rz")
                nc.vector.tensor_add(t_rz[:, 0:G], psum_rz[:, 0:G],
                                     b_rz[:, s:s + G])
                rs = epool.tile([128, GMAX], F32, tag="rs")
                nc.scalar.activation(rs[64:128, 0:G], t_rz[0:64, 0:G],
                                     AF.Sigmoid)
                zs = epool.tile([128, GMAX], F32, tag="zs")
                nc.scalar.activation(zs[64:128, 0:G], t_rz[64:128, 0:G],
                                     AF.Sigmoid)
                nc.vector.tensor_mul(rhs2[64:128, s:s + G], rs[64:128, 0:G],
                                     h_f[64:128, s:s + G])
                return w, zs

            def c_pass(k, s, G, w, zs):
                psum_c = pc_pool.tile([128, GMAX], F32, tag="c")
                cbase = G * 128
                for t in range(G // 2):
                    lw = w[:, cbase + t * 128:cbase + (t + 1) * 128]
                    nc.tensor.matmul(
                        psum_c[:, 2 * t:2 * t + 1], lw,
                        rhs2[:, s + 2 * t:s + 2 * t + 1],
                        start=True, stop=True,
                    )
                    nc.tensor.matmul(
                        psum_c[:, 2 * t + 1:2 * t + 2], lw,
                        rhs2[:, s + 2 * t + 1:s + 2 * t + 2],
                        start=True, stop=True,
                    )
                t_c = epool.tile([128, GMAX], F32, tag="t_c")
                nc.vector.tensor_add(t_c[:, 0:G], psum_c[:, 0:G],
                                     bc2[:, s:s + G])
                # even items' c sits in rows 0:64, odd items' in 64:128
                hc = epool.tile([128, GMAX], F32, tag="hc")
                nc.scalar.activation(hc[64:128, 0:G:2], t_c[0:64, 0:G:2],
                                     AF.Tanh)
                nc.scalar.activation(hc[64:128, 1:G:2], t_c[64:128, 1:G:2],
                                     AF.Tanh)
                # h_new = h + z*(hc - h)
                diff = epool.tile([128, GMAX], F32, tag="diff")
                nc.vector.tensor_sub(diff[64:128, 0:G], hc[64:128, 0:G],
                                     h_f[64:128, s:s + G])
                prod = epool.tile([128, GMAX], F32, tag="prod")
                nc.vector.tensor_mul(prod[64:128, 0:G], zs[64:128, 0:G],
                                     diff[64:128, 0:G])
                nc.vector.tensor_add(hn[64:128, s:s + G],
                                     h_f[64:128, s:s + G],
                                     prod[64:128, 0:G])
                # mid-kernel stores ride SWDGE: a store's semaphore wait on
                # the sync/scalar ENGINE would stall the next weight-DMA
                # issue.  Only the final store takes the (empty-by-then)
                # sync queue to dodge SWDGE's ~3us issue+completion latency.
                if k == NCHUNK - 1:
                    nc.sync.dma_start(out=out_d[:, s:s + G],
                                      in_=hn[64:128, s:s + G])
                else:
                    nc.gpsimd.dma_start(out=out_d[:, s:s + G],
                                        in_=hn[64:128, s:s + G])

            s = 0
            for k in range(NCHUNK):
                G = CHUNKS[k]
                state = rz_pass(k, s, G)
                if pending is not None:
                    c_pass(*pending)
                pending = (k, s, G) + state
                s += G
            c_pass(*pending)

    nc.compile()
    return nc


_CACHE = {}


def _get_nc():
    if "nc" not in _CACHE:
        _CACHE["nc"] = build_nc()
    return _CACHE["nc"]


def _pack(x, state, Wx, Wh, b):
    import ml_dtypes
    BF = ml_dtypes.bfloat16
    x2 = np.asarray(x, np.float32).reshape(ITEMS, C)
    h2 = np.asarray(state, np.float32).reshape(ITEMS, H)
    b2 = np.asarray(b, np.float32).reshape(ITEMS, J)
    wx = np.asarray(Wx, np.float32).reshape(ITEMS, C, J)
    wh = np.asarray(Wh, np.float32).reshape(ITEMS, H, J)
    w2 = np.concatenate([wx, wh], axis=1).astype(BF)   # [ITEMS, 128, 192]
    maps = []
    for i in range(NCORES):
        sl = slice(i * PER, (i + 1) * PER)
        xi, hi, bi, wi = x2[sl], h2[sl], b2[sl], w2[sl]

        xhx = np.zeros((128, 2 * PER), BF)
        xhx[0:64, 0:PER] = xi.T
        xhx[64:128, 0:PER] = hi.T
        xhx[0:64, PER:2 * PER] = xi.T

        auxf = np.zeros((128, 3 * PER), np.float32)
        auxf[64:128, 0:PER] = hi.T
        auxf[:, PER:2 * PER] = bi[:, 0:128].T
        bc = bi[:, 128:192].T                    # [64, PER]
        auxf[0:64, 2 * PER + 0:3 * PER:2] = bc[:, 0::2]
        auxf[64:128, 2 * PER + 1:3 * PER:2] = bc[:, 1::2]

        blocks = []
        s = 0
        for G in CHUNKS:
            wc = wi[s:s + G]                               # [G, 128, 192]
            rz = wc[:, :, 0:128].transpose(1, 0, 2).reshape(128, G * 128)
            cc = wc[:, :, 128:192].transpose(1, 0, 2).reshape(128, G * 64)
            blocks.append(
                np.ascontiguousarray(
                    np.concatenate([rz, cc], axis=1)).reshape(-1))
            s += G
        maps.append({"xhx": xhx, "auxf": auxf,
                     "wxh": np.concatenate(blocks)})
    return maps


def kernel(x, state, Wx, Wh, b, _trace=False):
    nc = _get_nc()
    in_maps = _pack(x, state, Wx, Wh, b)
    res = run_bass_kernel_spmd(nc, in_maps, list(range(NCORES)), trace=_trace)
    out = np.concatenate(
        [res.results[i]["out"].T for i in range(NCORES)], axis=0)
    ret = np.ascontiguousarray(out.reshape(B, N, 1, H), dtype=np.float32)
    if _trace:
        return ret, res
    return ret


# revision 10
# speedup vs baseline: 1.1153x; 1.1061x over previous
"""Trainium2 Bass kernel for batched per-item GRU cell (bf16 PE pipeline).

Problem: nn_GRU_Cell — B=16, N=207 independent items, each with its own
C=64 -> 3H=192 weight matrices (Wx, Wh).  All ops are per-(b,n):

    xW          = x @ Wx                      [1, 192]
    r           = sigmoid(xW_r + h @ Wh_r + b_r)
    z           = sigmoid(xW_z + h @ Wh_z + b_z)
    hc          = tanh  (xW_c + (r*h) @ Wh_c + b_c)
    h_new       = (1 - z) * h + z * hc

Strategy (per core, items sharded 3312 -> 8 x 414):
  * Weights are both the DMA and the PE bottleneck.  They stream ONCE as
    bf16 (halves HBM traffic vs f32, and bf16 stationaries load with the
    PE's Fast-Weight-Load at ~2x; f32 matmuls would also double-pass).
    Accuracy: bf16 weights/activations with f32 PSUM accumulate gives
    rel-err ~1.8e-3 on this problem (gate is 2e-2).
  * Per item the weights are the PE *stationary* operand, K-stacked:
      S_rz       = [Wx[:, 0:128] ; Wh[:, 0:128]]    (K=128, M=128)
      S_c (pair) = [Wc_even | Wc_odd], Wc = [Wx[:,128:192]; Wh[:,128:192]]
    The c-weights of two adjacent items are packed into ONE 128-column
    stationary so every LDWEIGHTS is a full 128-column load (FWL-eligible)
    and two c-matmuls share one weight load.
  * Moving operands are single bf16 columns:
      rz: [x ; h]     -> psum_rz[:, g]  (r rows 0:64, z rows 64:128)
      c : [x ; r*h]   -> psum_c [:, g]  (even items' c in rows 0:64,
                                         odd items' c in rows 64:128)
  * Biases are folded into PSUM by seeding each accumulation with an
    identity-stationary matmul over the bias panel (start=True), so the
    sigmoid/tanh ACT ops read PSUM directly — no DVE bias adds in the
    per-chunk serial chain.
  * All input transposition is done HOST-side: x/h/biases arrive as
    [feature, item] panels, so the kernel issues ZERO PE transposes.
    The output stays [H, items]; the host transposes it back.
  * The c-pass of chunk k is issued after the rz-pass of chunk k+1, so
    the PE never waits on the sigmoid/DVE producing the c moving operand.
"""

import numpy as np

import concourse.bass as bass
import concourse.mybir as mybir
import concourse.tile as tile
from concourse import bacc
from concourse.bass_utils import run_bass_kernel_spmd
from concourse.masks import make_identity

F32 = mybir.dt.float32
BF16 = mybir.dt.bfloat16
AF = mybir.ActivationFunctionType

B, N, C, H = 16, 207, 64, 64
J = 3 * H                  # 192
ITEMS = B * N              # 3312
NCORES = 8
PER = ITEMS // NCORES      # 414
# Small first chunk so the PE starts (and buffer recycling begins) early;
# small last chunks so the post-DMA serial tail is short.  All even.
CHUNKS = [16] + [52] * 7 + [22] + [12]   # sum = 414
NCHUNK = len(CHUNKS)
GMAX = max(CHUNKS)


def build_nc():
    nc = bacc.Bacc(None)
    # host-pre-transposed bf16 panels [128, 4*PER]:
    #   block0 = [x;h] (rz moving columns)
    #   block1 = [x;0] (c moving columns; rows 64:128 get r*h on-chip)
    #   block2 = b_rz^T
    #   block3 = b_c duplicated (even cols rows 0:64, odd cols rows 64:128)
    xhb_d = nc.declare_dram_parameter("xhb", [128, 4 * PER], BF16,
                                      isOutput=False)
    # f32 h panel for the exact h_new = h + z*(hc - h) update
    hf_d = nc.declare_dram_parameter("hf", [64, PER], F32, isOutput=False)
    # per-chunk blocks, each [c=128, G*192] flattened: G rz-stationaries
    # (128 cols each) then G/2 paired c-stationaries (128 cols each)
    w_d = nc.declare_dram_parameter("wxh", [PER * 2 * C * J], BF16,
                                    isOutput=False)
    out_d = nc.declare_dram_parameter("out", [64, PER], F32, isOutput=True)

    with tile.TileContext(nc) as tc:
        with (
            tc.tile_pool(name="const", bufs=1) as cpool,
            tc.tile_pool(name="w", bufs=6) as wpool,
            tc.tile_pool(name="ep", bufs=2) as epool,
            tc.tile_pool(name="prz", bufs=3, space="PSUM") as prz_pool,
            tc.tile_pool(name="pc", bufs=3, space="PSUM") as pc_pool,
        ):
            # one small preload leads each HWDGE queue (SWDGE would crawl
            # at ~65GB/s and burn ~25us of GpSimd Q7 descriptor generation)
            xhb = cpool.tile([128, 4 * PER], BF16)
            nc.sync.dma_start(out=xhb[:], in_=xhb_d[:])
            hpan = cpool.tile([128, PER], F32)
            nc.scalar.dma_start(out=hpan[64:128, :], in_=hf_d[:])
            ident = cpool.tile([128, 128], BF16)
            make_identity(nc, ident[:])
            hn = cpool.tile([128, PER], F32)

            b_rz = xhb[:, 2 * PER:3 * PER]
            bc2 = xhb[:, 3 * PER:4 * PER]
            rhs2 = xhb[:, PER:2 * PER]         # rows 0:64 = x (bf16)

            # software pipeline state: chunk k's c-pass+epilogue runs after
            # chunk k+1's rz-pass so PE never stalls on the sigmoid chain
            pending = None

            def rz_pass(k, s, G):
                w = wpool.tile([128, GMAX * J], BF16, tag="w")
                wq = nc.sync if k % 2 == 0 else nc.scalar
                wq.dma_start(
                    out=w[:, 0:G * J],
                    in_=w_d[s * 128 * J:(s + G) * 128 * J].rearrange(
                        "(c v) -> c v", c=128),
                )
                psum_rz = prz_pool.tile([128, GMAX], F32, tag="rz")
                # seed PSUM with b_rz (identity stationary), then accumulate
                nc.tensor.matmul(psum_rz[:, 0:G], ident[:],
                                 b_rz[:, s:s + G],
                                 start=True, stop=False,
                                 skip_group_check=True)
                for g in range(G):
                    nc.tensor.matmul(
                        psum_rz[:, g:g + 1],
                        w[:, g * 128:(g + 1) * 128],
                        xhb[:, s + g:s + g + 1],
                        start=False, stop=(g == G - 1),
                        skip_group_check=True,
                    )
                # r/z straight from PSUM (ACT reads PSUM fast)
                rs = epool.tile([128, GMAX], F32, tag="rs")
                nc.scalar.activation(rs[64:128, 0:G], psum_rz[0:64, 0:G],
                                     AF.Sigmoid)
                zs = epool.tile([128, GMAX], F32, tag="zs")
                nc.scalar.activation(zs[64:128, 0:G], psum_rz[64:128, 0:G],
                                     AF.Sigmoid)
                nc.vector.tensor_mul(rhs2[64:128, s:s + G], rs[64:128, 0:G],
                                     hpan[64:128, s:s + G])
                return w, zs

            def c_pass(k, s, G, w, zs):
                psum_c = pc_pool.tile([128, GMAX], F32, tag="c")
                nc.tensor.matmul(psum_c[:, 0:G], ident[:], bc2[:, s:s + G],
                                 start=True, stop=False,
                                 skip_group_check=True)
                cbase = G * 128
                for t in range(G // 2):
                    lw = w[:, cbase + t * 128:cbase + (t + 1) * 128]
                    nc.tensor.matmul(
                        psum_c[:, 2 * t:2 * t + 1], lw,
                        rhs2[:, s + 2 * t:s + 2 * t + 1],
                        start=False, stop=False,
                        skip_group_check=True,
                    )
                    nc.tensor.matmul(
                        psum_c[:, 2 * t + 1:2 * t + 2], lw,
                        rhs2[:, s + 2 * t + 1:s + 2 * t + 2],
                        start=False, stop=(t == G // 2 - 1),
                        skip_group_check=True,
                    )
                # even items' c sits in rows 0:64, odd items' in 64:128
                hc = epool.tile([128, GMAX], F32, tag="hc")
                nc.scalar.activation(hc[64:128, 0:G:2], psum_c[0:64, 0:G:2],
                                     AF.Tanh)
                nc.scalar.activation(hc[64:128, 1:G:2],
                                     psum_c[64:128, 1:G:2], AF.Tanh)
                # h_new = h + z*(hc - h)
                diff = epool.tile([128, GMAX], F32, tag="diff")
                nc.vector.tensor_sub(diff[64:128, 0:G], hc[64:128, 0:G],
                                     hpan[64:128, s:s + G])
                prod = epool.tile([128, GMAX], F32, tag="prod")
                nc.vector.tensor_mul(prod[64:128, 0:G], zs[64:128, 0:G],
                                     diff[64:128, 0:G])
                nc.vector.tensor_add(hn[64:128, s:s + G],
                                     hpan[64:128, s:s + G],
                                     prod[64:128, 0:G])
                # mid-kernel stores ride SWDGE: a store's semaphore wait on
                # the sync/scalar ENGINE would stall the next weight-DMA
                # issue.  Only the final store takes the (empty-by-then)
                # sync queue to dodge SWDGE's ~3us issue+completion latency.
                if k == NCHUNK - 1:
                    nc.sync.dma_start(out=out_d[:, s:s + G],
                                      in_=hn[64:128, s:s + G])
                else:
                    nc.gpsimd.dma_start(out=out_d[:, s:s + G],
                                        in_=hn[64:128, s:s + G])

            s = 0
            for k in range(NCHUNK):
                G = CHUNKS[k]
                state = rz_pass(k, s, G)
                if pending is not None:
                    c_pass(*pending)
                pending = (k, s, G) + state
                s += G
            c_pass(*pending)

    nc.compile()
    return nc


_CACHE = {}


def _get_nc():
    if "nc" not in _CACHE:
        _CACHE["nc"] = build_nc()
    return _CACHE["nc"]


def _pack(x, state, Wx, Wh, b):
    import ml_dtypes
    BF = ml_dtypes.bfloat16
    x2 = np.asarray(x, np.float32).reshape(ITEMS, C)
    h2 = np.asarray(state, np.float32).reshape(ITEMS, H)
    b2 = np.asarray(b, np.float32).reshape(ITEMS, J)
    wx = np.asarray(Wx, np.float32).reshape(ITEMS, C, J)
    wh = np.asarray(Wh, np.float32).reshape(ITEMS, H, J)
    w2 = np.concatenate([wx, wh], axis=1).astype(BF)   # [ITEMS, 128, 192]
    maps = []
    for i in range(NCORES):
        sl = slice(i * PER, (i + 1) * PER)
        xi, hi, bi, wi = x2[sl], h2[sl], b2[sl], w2[sl]

        xhb = np.zeros((128, 4 * PER), BF)
        xhb[0:64, 0:PER] = xi.T
        xhb[64:128, 0:PER] = hi.T
        xhb[0:64, PER:2 * PER] = xi.T
        xhb[:, 2 * PER:3 * PER] = bi[:, 0:128].T
        bc = bi[:, 128:192].T                    # [64, PER]
        xhb[0:64, 3 * PER + 0:4 * PER:2] = bc[:, 0::2]
        xhb[64:128, 3 * PER + 1:4 * PER:2] = bc[:, 1::2]

        blocks = []
        s = 0
        for G in CHUNKS:
            wc = wi[s:s + G]                               # [G, 128, 192]
            rz = wc[:, :, 0:128].transpose(1, 0, 2).reshape(128, G * 128)
            cc = wc[:, :, 128:192].transpose(1, 0, 2).reshape(128, G * 64)
            blocks.append(
                np.ascontiguousarray(
                    np.concatenate([rz, cc], axis=1)).reshape(-1))
            s += G
        maps.append({"xhb": xhb, "hf": np.ascontiguousarray(hi.T),
                     "wxh": np.concatenate(blocks)})
    return maps


def kernel(x, state, Wx, Wh, b, _trace=False):
    nc = _get_nc()
    in_maps = _pack(x, state, Wx, Wh, b)
    res = run_bass_kernel_spmd(nc, in_maps, list(range(NCORES)), trace=_trace)
    out = np.concatenate(
        [res.results[i]["out"].T for i in range(NCORES)], axis=0)
    ret = np.ascontiguousarray(out.reshape(B, N, 1, H), dtype=np.float32)
    if _trace:
        return ret, res
    return ret
